# revision 14
# baseline (speedup 1.0000x reference)
"""Cosine-similarity attention map on 8 Trainium2 NeuronCores.

out[b, i, j] = <x[b,:,i], x[b,:,j]> / (||x[b,:,i]|| * ||x[b,:,j]||)
x: [B=4, C=64, N=4096] fp32  ->  out: [B=4, N=4096, N=4096] fp32

The output is a symmetric Gram matrix per batch, so each core only
computes a unique half of it (SYRK-style) and the host mirrors the rest
while unsharding. Sharding: 2 cores per batch running the SAME program;
core (b, 0) gets x[b], core (b, 1) gets x[b] with columns reversed.
In its own index space every core computes, for each 128-row tile
a in [0,16): cols [128a, 2048) (triangle part, "R1") and cols
[3968-128a, 4096) (anti-diagonal cross part, "R2") -- a constant 2176
columns per tile. The identity-core blocks plus the mirrored
reversed-core blocks tile the full matrix exactly once (plus the 16
anti-diagonal blocks twice). Output is fp16 (tolerance is 2e-2; fp16
adds ~3e-4), upcast on the host: 8.9 MiB of HBM writes per core.

PE-array tiling: K=C=64 only fills half the 128x128 array, so the
normalized operands are kept TWICE -- cols [0,2048) on SBUF partitions
0-63 and cols [2048,4096) plus a copy of the stationary cols on
partitions 64-127. R1 matmuls run on PE row-group 0, R2 matmuls on
row-group 64 (tile_position), and the two streams execute
concurrently, halving the PE wall time. Tile emission order interleaves
big-R1 and big-R2 tiles to keep both row-groups fed. The norm pipeline
(square -> ones[64,64]-matmul reduce+broadcast -> Abs_reciprocal_sqrt
-> y = x*rsqrt) runs per-half on its own partitions; the rsqrt values
for the stationary cols are relayed to the upper half with a small
SBUF->SBUF DMA.
"""

import sys

sys.path.insert(0, "/opt/trn_rl_repo")

import numpy as np

import concourse.bass as bass
import concourse.mybir as mybir
import concourse.tile as tile
from concourse import bacc
from concourse.bass_utils import run_bass_kernel_spmd
from concourse.vector_clock import ScopedClock, VectorClock

B, C, N = 4, 64, 4096
NCORES = 8
NTILES = 16  # 128-row output tiles per core
TW = 2176  # output columns per row tile (constant by construction)
UW = 2 * TW  # two tiles share one 128-row panel
# Interleave big-R1 (low a) and big-R2 (high a) tiles so the two PE
# row-group streams stay balanced; a couple of R1-heavy tiles first to
# cover the upper half's extra norm latency.
TILE_ORDER = [0, 1, 2, 3, 15, 4, 14, 5, 13, 6, 12, 7, 11, 8, 10, 9]

F32 = mybir.dt.float32
F16 = mybir.dt.float16
AbsRsqrt = mybir.ActivationFunctionType.Abs_reciprocal_sqrt


class SplitDrainTileContext(tile.TileContext):
    """Stock TileContext attaches a wait for every pending DMA-queue
    semaphore to a single exit Drain; the walrus build here only allows one
    sync-wait per TPB_CTRL instruction ("Too many sync wait commands").
    Emit one drain per pending logical processor instead."""

    def _drain_and_barrier(self, tick_clock, wait_clock):
        gc = tick_clock.global_clock
        n = len(gc)
        for p in range(n):
            t = gc[p]
            if t <= 0:
                continue
            part = VectorClock([t if q == p else 0 for q in range(n)])
            d = self.nc.sync.drain()
            wait_clock.add_sem_waits(d.ins, ScopedClock({None: part}))

        self.nc.all_engine_barrier()
        assert self.sems is not None
        popped = self.nc._tile_sem_poison_stack.pop()
        assert popped is self._sem_poison
        self.nc.clear_and_free_semaphores(list(self.sems.allocated().values()))
        self.nc.all_engine_barrier()


def _ranges(a):
    """(start, width) column ranges of row tile a: triangle + cross part."""
    return [(128 * a, 2048 - 128 * a), (3968 - 128 * a, 128 + 128 * a)]


def _pack(ranges):
    """Pack matmul chunks into [128,1024] PSUM pair-tiles.

    Returns groups: (mms, glen) where mms = [(rhs_start, width, slot_off)].
    Each matmul stays within one 512-col PSUM bank; each group is copied
    to SBUF with a single contiguous [0:glen) copy.
    """
    groups, cur, cur_len = [], [], 0
    for start, width in ranges:
        done = 0
        while done < width:
            if cur_len == 1024:
                groups.append((cur, cur_len))
                cur, cur_len = [], 0
            w = min(512 - (cur_len % 512), width - done)
            cur.append((start + done, w, cur_len))
            cur_len += w
            done += w
    if cur:
        groups.append((cur, cur_len))
    return groups


def _build(use_split_drain=False):
    nc = bacc.Bacc("TRN2", target_bir_lowering=False)
    xf = nc.declare_dram_parameter("xf", [C, N], F32, isOutput=False)
    out = nc.declare_dram_parameter("out", [NTILES // 2 * 128, UW], F16, isOutput=True)

    tc_cls = SplitDrainTileContext if use_split_drain else tile.TileContext
    with tc_cls(nc) as tc:
        with (
            tc.tile_pool(name="persist", bufs=1) as persist,
            tc.tile_pool(name="panels", bufs=3) as panels,
            tc.tile_pool(name="mpsum", bufs=3, space="PSUM") as mpsum,
            tc.tile_pool(name="npsum", bufs=1, space="PSUM") as npsum,
        ):
            # Lower partitions (0-63): x cols [0,2048). Upper partitions
            # (64-127): the full x -- [2048,4096) feeds the R2 norm, and
            # [0,2048) feeds the upper copy of the stationary operand.
            XB = persist.tile([128, N], F32)
            nc.sync.dma_start(out=XB[0:C, 0:1024], in_=xf[:, 0:1024])
            nc.sync.dma_start(out=XB[0:C, 1024:2048], in_=xf[:, 1024:2048])
            nc.sync.dma_start(out=XB[C:128, 2048:3072], in_=xf[:, 2048:3072])
            nc.sync.dma_start(out=XB[C:128, 3072:N], in_=xf[:, 3072:N])

            ones_f = persist.tile([128, C], F32)
            nc.vector.memset(ones_f, 1.0)
            ones_J = persist.tile([128, C], F16)  # reduce+broadcast lhsT
            nc.vector.tensor_copy(ones_J, ones_f)

            SQB = persist.tile([128, N], F16)
            RNB = persist.tile([128, N], F16)
            YB = persist.tile([128, N], F16)

            def norm_unit(p):
                # Normalize cols [1024p, 1024p+1024) on its own partition
                # half: square -> ones[64,64]-matmul = column sums
                # broadcast to the half's 64 partitions (PE row/col group,
                # one matmul per 512) -> 1/sqrt straight off PSUM to fp16
                # (ACT) -> y = x*rsqrt (DVE). GpSimd crashes the exec unit
                # on partition offsets >= 64, so it only touches lower data.
                lo = p < 2
                ps_ = slice(0, C) if lo else slice(C, 128)
                tp = (0, 0) if lo else (C, C)
                cs = slice(1024 * p, 1024 * (p + 1))
                if p == 0:
                    nc.gpsimd.tensor_mul(SQB[ps_, cs], XB[ps_, cs], XB[ps_, cs])
                elif p == 2:
                    nc.vector.tensor_mul(SQB[ps_, cs], XB[ps_, cs], XB[ps_, cs])
                else:
                    nc.scalar.square(SQB[ps_, cs], XB[ps_, cs])
                pj = npsum.tile([128, 1024], F32, tag="pj")
                for h in range(2):
                    nc.tensor.matmul(
                        pj[ps_, 512 * h : 512 * h + 512],
                        lhsT=ones_J[ps_, :],
                        rhs=SQB[ps_, 1024 * p + 512 * h : 1024 * p + 512 * (h + 1)],
                        start=True,
                        stop=True,
                        tile_position=tp,
                    )
                nc.scalar.activation(RNB[ps_, cs], pj[ps_, :], AbsRsqrt)
                nc.vector.tensor_mul(YB[ps_, cs], XB[ps_, cs], RNB[ps_, cs])

            ncopy = 0

            def emit_tile(a, panel, off):
                # SYRK matmuls for row tile a, packed into PSUM pair-tiles.
                # R1 chunks (cols < 2048) use the lower operands on PE
                # row-group 0; R2 chunks (cols >= 2048) use the upper
                # operands on row-group 64 -- the two streams execute
                # concurrently in the array. One contiguous PSUM->SBUF fp16
                # copy per pair-tile, alternating DVE / ACT.
                nonlocal ncopy
                r1, r2 = _ranges(a)
                # R1 (row-group 0) and R2 (row-group 64) never share a PSUM
                # bank: concurrent row-group matmuls into one bank hang the
                # exec unit.
                for rng in (r1, r2):
                    lo = rng[0] < 2048
                    ps_ = slice(0, C) if lo else slice(C, 128)
                    for mms, glen in _pack([rng]):
                        ps = mpsum.tile([128, 1024], F32, tag="ps")
                        for rs, w, so in mms:
                            nc.tensor.matmul(
                                ps[:, so : so + w],
                                lhsT=YB[ps_, 128 * a : 128 * a + 128],
                                rhs=YB[ps_, rs : rs + w],
                                start=True,
                                stop=True,
                                tile_position=(0 if lo else C, 0),
                            )
                        if ncopy % 5 in (0, 3):
                            nc.vector.tensor_copy(
                                panel[:, off : off + glen], ps[:, :glen]
                            )
                        else:
                            nc.scalar.copy(
                                out=panel[:, off : off + glen], in_=ps[:, :glen]
                            )
                        ncopy += 1
                        off += glen

            # Lower-half norm first (R1 operands); the finished y for the
            # stationary cols is relayed to the upper partitions with a
            # SBUF->SBUF DMA. Upper-half norm (R2 operands) overlaps the
            # first SYRK tiles.
            for p in range(2):
                norm_unit(p)
                cs = slice(1024 * p, 1024 * (p + 1))
                nc.sync.dma_start(out=YB[C:128, cs], in_=YB[0:C, cs])
            for p in range(2, 4):
                norm_unit(p)

            for idx, a in enumerate(TILE_ORDER):
                u, s = divmod(idx, 2)
                if s == 0:
                    panel = panels.tile([128, UW], F16, tag="panel")
                emit_tile(a, panel, s * TW)
                nc.sync.dma_start(
                    out=out[u * 128 : (u + 1) * 128, s * TW : (s + 1) * TW],
                    in_=panel[:, s * TW : (s + 1) * TW],
                )

    nc.compile()
    return nc


def _install_profile_hook():
    """This container's antenv lacks axon_hooks, so run_bass_kernel_spmd's
    trace=True path dies on import. Recreate the module and register the
    ctypes NTFF hook that trn_boot would have installed."""
    import sys as _sys
    import types

    if "antenv.axon_hooks" in _sys.modules:
        return
    import antenv

    mod = types.ModuleType("antenv.axon_hooks")
    mod._hook = None

    def set_axon_ntff_profile_hook(h):
        mod._hook = h

    def get_axon_ntff_profile_hook():
        return mod._hook

    mod.set_axon_ntff_profile_hook = set_axon_ntff_profile_hook
    mod.get_axon_ntff_profile_hook = get_axon_ntff_profile_hook
    _sys.modules["antenv.axon_hooks"] = mod
    antenv.axon_hooks = mod

    from trn_agent_boot.trn_boot import _ntff_profile_via_ctypes

    mod.set_axon_ntff_profile_hook(
        _ntff_profile_via_ctypes("/opt/axon/libaxon_pjrt.so")
    )


_nc = None


def _get_nc():
    global _nc
    if _nc is None:
        _nc = _build()
    return _nc


# Ordered output blocks (32x32 grid of 128x128) filled by the two cores
# of a batch; the rest is mirrored from the transpose on the host.
_FILLED = np.zeros((32, 32), bool)
for _a in range(16):
    _FILLED[_a, _a:16] = True
    _FILLED[_a, 31 - _a : 32] = True
    _FILLED[31 - _a, 16 : 32 - _a] = True
    _FILLED[31 - _a, 0 : _a + 1] = True
_MISS_I, _MISS_J = np.nonzero(~_FILLED)


def _run(x, trace=False, trace_cores=None):
    x = np.asarray(x, dtype=np.float32)
    assert x.shape == (B, C, N), x.shape
    core_ids = list(range(NCORES))
    in_maps = []
    for k in core_ids:
        b, half = divmod(k, 2)
        xb = x[b] if half == 0 else x[b][:, ::-1]
        in_maps.append({"xf": np.ascontiguousarray(xb)})
    if trace:
        _install_profile_hook()
    res = run_bass_kernel_spmd(
        _get_nc(), in_maps, core_ids, trace=trace, trace_cores=trace_cores
    )
    out = np.empty((B, N, N), dtype=np.float32)
    for k in core_ids:
        b, half = divmod(k, 2)
        O = res.results[k]["out"]
        M = out[b]
        for idx, a in enumerate(TILE_ORDER):
            u, s = divmod(idx, 2)
            P = O[u * 128 : (u + 1) * 128, s * TW : (s + 1) * TW]
            W1 = 2048 - 128 * a
            if half == 0:
                M[128 * a : 128 * a + 128, 128 * a : 2048] = P[:, :W1]
                M[128 * a : 128 * a + 128, 3968 - 128 * a : 4096] = P[:, W1:]
            else:
                M[3968 - 128 * a : 4096 - 128 * a, 2048 : 4096 - 128 * a] = P[
                    :, :W1
                ][::-1, ::-1]
                M[3968 - 128 * a : 4096 - 128 * a, 0 : 128 * a + 128] = P[:, W1:][
                    ::-1, ::-1
                ]
    for b in range(B):
        Mb = out[b].reshape(32, 128, 32, 128)
        Mb[_MISS_I, :, _MISS_J, :] = Mb[_MISS_J, :, _MISS_I, :].transpose(0, 2, 1)
    return out, res


def kernel(x):
    return _run(x)[0]


# revision 15
# speedup vs baseline: 1.2724x; 1.2724x over previous
"""Cosine-similarity attention map on 8 Trainium2 NeuronCores.

out[b, i, j] = <x[b,:,i], x[b,:,j]> / (||x[b,:,i]|| * ||x[b,:,j]||)
x: [B=4, C=64, N=4096] fp32  ->  out: [B=4, N=4096, N=4096] fp32

The output is a symmetric Gram matrix per batch, so each core only
computes a unique half of it (SYRK-style) and the host mirrors the rest
while unsharding. Sharding: 2 cores per batch running the SAME program;
core (b, 0) gets x[b], core (b, 1) gets x[b] with columns reversed.
In its own index space every core computes, for each 128-row tile
a in [0,16): cols [128a, 2048) (triangle part, "R1") and cols
[3968-128a, 4096) (anti-diagonal cross part, "R2") -- a constant 2176
columns per tile. The identity-core blocks plus the mirrored
reversed-core blocks tile the full matrix exactly once (plus the 16
anti-diagonal blocks twice). Output is fp16 (tolerance is 2e-2; fp16
adds ~3e-4), upcast on the host: 8.9 MiB of HBM writes per core.

PE-array tiling: K=C=64 only fills half the 128x128 array, so the
normalized operands are kept TWICE -- cols [0,2048) on SBUF partitions
0-63 and cols [2048,4096) plus a copy of the stationary cols on
partitions 64-127. R1 matmuls run on PE row-group 0, R2 matmuls on
row-group 64 (tile_position), and the two streams execute
concurrently, halving the PE wall time. Tile emission order interleaves
big-R1 and big-R2 tiles to keep both row-groups fed. The norm pipeline
(square -> ones[64,64]-matmul reduce+broadcast -> Abs_reciprocal_sqrt
-> y = x*rsqrt) runs per-half on its own partitions; the rsqrt values
for the stationary cols are relayed to the upper half with a small
SBUF->SBUF DMA.
"""

import sys

sys.path.insert(0, "/opt/trn_rl_repo")

import numpy as np

import concourse.bass as bass
import concourse.mybir as mybir
import concourse.tile as tile
from concourse import bacc
from concourse.bass_utils import run_bass_kernel_spmd
from concourse.vector_clock import ScopedClock, VectorClock

B, C, N = 4, 64, 4096
NCORES = 8
NTILES = 16  # 128-row output tiles per core
TW = 2176  # output columns per row tile (constant by construction)
UW = 2 * TW  # two tiles share one 128-row panel
# Interleave big-R1 (low a) and big-R2 (high a) tiles so the two PE
# row-group streams stay balanced; a couple of R1-heavy tiles first to
# cover the upper half's extra norm latency.
TILE_ORDER = [0, 1, 2, 3, 15, 4, 14, 5, 13, 6, 12, 7, 11, 8, 10, 9]

F32 = mybir.dt.float32
F16 = mybir.dt.float16
AbsRsqrt = mybir.ActivationFunctionType.Abs_reciprocal_sqrt


class SplitDrainTileContext(tile.TileContext):
    """Stock TileContext attaches a wait for every pending DMA-queue
    semaphore to a single exit Drain; the walrus build here only allows one
    sync-wait per TPB_CTRL instruction ("Too many sync wait commands").
    Emit one drain per pending logical processor instead."""

    def _drain_and_barrier(self, tick_clock, wait_clock):
        gc = tick_clock.global_clock
        n = len(gc)
        for p in range(n):
            t = gc[p]
            if t <= 0:
                continue
            part = VectorClock([t if q == p else 0 for q in range(n)])
            d = self.nc.sync.drain()
            wait_clock.add_sem_waits(d.ins, ScopedClock({None: part}))

        self.nc.all_engine_barrier()
        assert self.sems is not None
        popped = self.nc._tile_sem_poison_stack.pop()
        assert popped is self._sem_poison
        self.nc.clear_and_free_semaphores(list(self.sems.allocated().values()))
        self.nc.all_engine_barrier()


def _ranges(a):
    """(start, width) column ranges of row tile a: triangle + cross part."""
    return [(128 * a, 2048 - 128 * a), (3968 - 128 * a, 128 + 128 * a)]


def _pack(ranges):
    """Pack matmul chunks into [128,1024] PSUM pair-tiles.

    Returns groups: (mms, glen) where mms = [(rhs_start, width, slot_off)].
    Each matmul stays within one 512-col PSUM bank; each group is copied
    to SBUF with a single contiguous [0:glen) copy.
    """
    groups, cur, cur_len = [], [], 0
    for start, width in ranges:
        done = 0
        while done < width:
            if cur_len == 1024:
                groups.append((cur, cur_len))
                cur, cur_len = [], 0
            w = min(512 - (cur_len % 512), width - done)
            cur.append((start + done, w, cur_len))
            cur_len += w
            done += w
    if cur:
        groups.append((cur, cur_len))
    return groups


def _build(use_split_drain=False):
    nc = bacc.Bacc("TRN2", target_bir_lowering=False)
    xf = nc.declare_dram_parameter("xf", [C, N], F32, isOutput=False)
    out = nc.declare_dram_parameter("out", [NTILES // 2 * 128, UW], F16, isOutput=True)

    tc_cls = SplitDrainTileContext if use_split_drain else tile.TileContext
    with tc_cls(nc) as tc:
        with (
            tc.tile_pool(name="persist", bufs=1) as persist,
            tc.tile_pool(name="panels", bufs=3) as panels,
            tc.tile_pool(name="mpsum", bufs=4, space="PSUM") as mpsum,
        ):
            # Lower partitions (0-63): x cols [0,2048). Upper partitions
            # (64-127): the full x -- [2048,4096) feeds the R2 norm, and
            # [0,2048) feeds the upper copy of the stationary operand.
            XB = persist.tile([128, N], F32)
            nc.sync.dma_start(out=XB[0:C, 0:512], in_=xf[:, 0:512])
            nc.sync.dma_start(out=XB[0:C, 512:1024], in_=xf[:, 512:1024])
            nc.sync.dma_start(out=XB[0:C, 1024:2048], in_=xf[:, 1024:2048])
            nc.sync.dma_start(out=XB[C:128, 2048:3072], in_=xf[:, 2048:3072])
            nc.sync.dma_start(out=XB[C:128, 3072:N], in_=xf[:, 3072:N])

            ones_f = persist.tile([128, C], F32)
            nc.vector.memset(ones_f, 1.0)
            ones_J = persist.tile([128, C], F16)  # reduce+broadcast lhsT
            nc.vector.tensor_copy(ones_J, ones_f)

            SQB = persist.tile([128, N], F16)
            RNB = persist.tile([128, N], F16)
            YB = persist.tile([128, N], F16)

            def norm_span(c0, w, sq_eng):
                # Normalize cols [c0, c0+w) on the half owning them:
                # square -> ones[64,64]-matmul = column sums broadcast to
                # the half's 64 partitions (PE row/col group, one matmul
                # per 512) -> 1/sqrt straight off PSUM to fp16 (ACT) ->
                # y = x*rsqrt (DVE). The lower half runs at 512 cols for
                # latency (it gates the first SYRK tiles), the upper at
                # 1024. GpSimd crashes the exec unit on partition offsets
                # >= 64, so it only ever touches lower-half data.
                lo = c0 < 2048
                ps_ = slice(0, C) if lo else slice(C, 128)
                cs = slice(c0, c0 + w)
                if sq_eng is nc.scalar:
                    nc.scalar.square(SQB[ps_, cs], XB[ps_, cs])
                else:
                    sq_eng.tensor_mul(SQB[ps_, cs], XB[ps_, cs], XB[ps_, cs])
                pj = mpsum.tile([128, 1024], F32, tag="ps")
                for h0 in range(0, w, 512):
                    nc.tensor.matmul(
                        pj[ps_, h0 : h0 + 512],
                        lhsT=ones_J[ps_, :],
                        rhs=SQB[ps_, c0 + h0 : c0 + h0 + 512],
                        start=True,
                        stop=True,
                        tile_position=(0, 0) if lo else (C, C),
                    )
                nc.scalar.activation(RNB[ps_, cs], pj[ps_, 0:w], AbsRsqrt)
                nc.vector.tensor_mul(YB[ps_, cs], XB[ps_, cs], RNB[ps_, cs])

            ncopy = 0

            def emit_tile(a, panel, off):
                # SYRK matmuls for row tile a, packed into PSUM pair-tiles.
                # R1 chunks (cols < 2048) use the lower operands on PE
                # row-group 0; R2 chunks (cols >= 2048) use the upper
                # operands on row-group 64 -- the two streams execute
                # concurrently in the array. One contiguous PSUM->SBUF fp16
                # copy per pair-tile, alternating DVE / ACT.
                nonlocal ncopy
                r1, r2 = _ranges(a)
                # R1 (row-group 0) and R2 (row-group 64) never share a PSUM
                # bank: concurrent row-group matmuls into one bank hang the
                # exec unit.
                for rng in (r1, r2):
                    lo = rng[0] < 2048
                    ps_ = slice(0, C) if lo else slice(C, 128)
                    for mms, glen in _pack([rng]):
                        ps = mpsum.tile([128, 1024], F32, tag="ps")
                        for rs, w, so in mms:
                            nc.tensor.matmul(
                                ps[:, so : so + w],
                                lhsT=YB[ps_, 128 * a : 128 * a + 128],
                                rhs=YB[ps_, rs : rs + w],
                                start=True,
                                stop=True,
                                tile_position=(0 if lo else C, 0),
                            )
                        if ncopy % 2 == 0:
                            nc.vector.tensor_copy(
                                panel[:, off : off + glen], ps[:, :glen]
                            )
                        else:
                            nc.scalar.copy(
                                out=panel[:, off : off + glen], in_=ps[:, :glen]
                            )
                        ncopy += 1
                        off += glen

            # Lower-half norm first (R1 operands); the finished y for the
            # stationary cols is relayed to the upper partitions with a
            # SBUF->SBUF DMA. Upper-half norm (R2 operands) overlaps the
            # first SYRK tiles.
            for c, eng in enumerate((nc.vector, nc.gpsimd, nc.scalar, nc.vector)):
                norm_span(512 * c, 512, eng)
                cs = slice(512 * c, 512 * (c + 1))
                nc.sync.dma_start(out=YB[C:128, cs], in_=YB[0:C, cs])
            norm_span(2048, 1024, nc.vector)
            norm_span(3072, 1024, nc.scalar)

            for idx, a in enumerate(TILE_ORDER):
                u, s = divmod(idx, 2)
                if s == 0:
                    panel = panels.tile([128, UW], F16, tag="panel")
                emit_tile(a, panel, s * TW)
                nc.sync.dma_start(
                    out=out[u * 128 : (u + 1) * 128, s * TW : (s + 1) * TW],
                    in_=panel[:, s * TW : (s + 1) * TW],
                )

    nc.compile()
    return nc


def _install_profile_hook():
    """This container's antenv lacks axon_hooks, so run_bass_kernel_spmd's
    trace=True path dies on import. Recreate the module and register the
    ctypes NTFF hook that trn_boot would have installed."""
    import sys as _sys
    import types

    if "antenv.axon_hooks" in _sys.modules:
        return
    import antenv

    mod = types.ModuleType("antenv.axon_hooks")
    mod._hook = None

    def set_axon_ntff_profile_hook(h):
        mod._hook = h

    def get_axon_ntff_profile_hook():
        return mod._hook

    mod.set_axon_ntff_profile_hook = set_axon_ntff_profile_hook
    mod.get_axon_ntff_profile_hook = get_axon_ntff_profile_hook
    _sys.modules["antenv.axon_hooks"] = mod
    antenv.axon_hooks = mod

    from trn_agent_boot.trn_boot import _ntff_profile_via_ctypes

    mod.set_axon_ntff_profile_hook(
        _ntff_profile_via_ctypes("/opt/axon/libaxon_pjrt.so")
    )


_nc = None


def _get_nc():
    global _nc
    if _nc is None:
        _nc = _build()
    return _nc


# Ordered output blocks (32x32 grid of 128x128) filled by the two cores
# of a batch; the rest is mirrored from the transpose on the host.
_FILLED = np.zeros((32, 32), bool)
for _a in range(16):
    _FILLED[_a, _a:16] = True
    _FILLED[_a, 31 - _a : 32] = True
    _FILLED[31 - _a, 16 : 32 - _a] = True
    _FILLED[31 - _a, 0 : _a + 1] = True
_MISS_I, _MISS_J = np.nonzero(~_FILLED)


def _run(x, trace=False, trace_cores=None):
    x = np.asarray(x, dtype=np.float32)
    assert x.shape == (B, C, N), x.shape
    core_ids = list(range(NCORES))
    in_maps = []
    for k in core_ids:
        b, half = divmod(k, 2)
        xb = x[b] if half == 0 else x[b][:, ::-1]
        in_maps.append({"xf": np.ascontiguousarray(xb)})
    if trace:
        _install_profile_hook()
    res = run_bass_kernel_spmd(
        _get_nc(), in_maps, core_ids, trace=trace, trace_cores=trace_cores
    )
    out = np.empty((B, N, N), dtype=np.float32)
    for k in core_ids:
        b, half = divmod(k, 2)
        O = res.results[k]["out"]
        M = out[b]
        for idx, a in enumerate(TILE_ORDER):
            u, s = divmod(idx, 2)
            P = O[u * 128 : (u + 1) * 128, s * TW : (s + 1) * TW]
            W1 = 2048 - 128 * a
            if half == 0:
                M[128 * a : 128 * a + 128, 128 * a : 2048] = P[:, :W1]
                M[128 * a : 128 * a + 128, 3968 - 128 * a : 4096] = P[:, W1:]
            else:
                M[3968 - 128 * a : 4096 - 128 * a, 2048 : 4096 - 128 * a] = P[
                    :, :W1
                ][::-1, ::-1]
                M[3968 - 128 * a : 4096 - 128 * a, 0 : 128 * a + 128] = P[:, W1:][
                    ::-1, ::-1
                ]
    for b in range(B):
        Mb = out[b].reshape(32, 128, 32, 128)
        Mb[_MISS_I, :, _MISS_J, :] = Mb[_MISS_J, :, _MISS_I, :].transpose(0, 2, 1)
    return out, res


def kernel(x):
    return _run(x)[0]
